# revision 6
# baseline (speedup 1.0000x reference)
"""Contrastive loss (SimCLR-style, B=1024, emb [1024,128,128]) on 8 TRN2 cores.

Strategy: shard the contraction dim D=16384 (= 128 m x 128 n, m-major) by
m-chunks of 16 across the 8 cores. Each core receives its chunk of both
embeddings pre-transposed and pre-quantized to fp8e4m3 in DoubleRow layout
x[k, n, s, r] = fp8(emb[r, 16c + 2k + s, n]), rows r = concat(i-batch,
j-batch).

Per core:
  1. partial sum-of-squares over local m -> 0.5 MiB bf16 AllReduce -> full
     per-(n, row) column norms -> scale = 64/sqrt(128*ssq) (the /sqrt(128)
     flat-row norm is exact: columns are unit after dim-1 normalize; the
     x64 prescale keeps fp8 operands in e4m3's normal range and is divided
     back out inside the loss exp/log constants).
  2. rn tiles (fp8, in-place) = x * scale.
  3. gram partial sim_c = rn_c^T rn_c [2048, 2048] f32 on PE with fp8
     DoubleRow (K=256 per instruction); partials sum across cores.
  4. four chunked bf16 ReduceScatters ([512, 2048] each) -> each core owns
     4 x 64 rows of the summed (x4096-scaled) sim.
  5. loss on owned rows: exp(sim/2048) row-sum (ACT accum), minus the
     self-sim term (eye mask), log, minus positives (pos mask) ->
     partition-sum via ones-matmul -> scalar.  Host sums 8 scalars / 2048.
"""

import numpy as np
import ml_dtypes

import concourse.bacc as bacc
import concourse.mybir as mybir
import concourse.tile as tile
from concourse import bass_utils

F32 = mybir.dt.float32
BF16 = mybir.dt.bfloat16
FP8 = mybir.dt.float8e4
AF = mybir.ActivationFunctionType
ALU = mybir.AluOpType
PM = mybir.MatmulPerfMode

B = 1024
R = 2 * B            # 2048 rows
NCORES = 8
KTILES = 8           # DoubleRow K-tiles per core (256 K each)
MT = R // 128        # 16 output row tiles
NRS = 4              # reduce-scatter chunks
S = 64.0             # fp8 prescale; sim comes out x S^2
INV_T_S2 = 2.0 / (S * S)   # 1/TEMP / S^2

_CACHE = {}


def _build_nc():
    if "nc" in _CACHE:
        return _CACHE["nc"]
    nc = bacc.Bacc("TRN2", target_bir_lowering=False, debug=False,
                   num_devices=NCORES)

    x = nc.dram_tensor("x", [KTILES, 128, 2 * R], FP8, kind="ExternalInput")
    masks = nc.dram_tensor("masks", [4, 128, R], BF16, kind="ExternalInput")
    y = nc.dram_tensor("y", [1, 1], F32, kind="ExternalOutput")

    cc_warm_in = nc.dram_tensor("cc_warm_in", [1, 16], F32)
    cc_warm_out = nc.dram_tensor("cc_warm_out", [1, 16], F32, addr_space="Shared")
    cc_ssq_in = nc.dram_tensor("cc_ssq_in", [128, R], BF16)
    cc_ssq_out = nc.dram_tensor("cc_ssq_out", [128, R], BF16, addr_space="Shared")
    rs_rows = R // NRS
    cc_sim_in = [nc.dram_tensor(f"cc_sim_in{i}", [rs_rows, R], BF16)
                 for i in range(NRS)]
    cc_rs = [nc.dram_tensor(f"cc_rs{i}", [rs_rows // NCORES, R], BF16)
             for i in range(NRS)]
    grp = [list(range(NCORES))]

    with tile.TileContext(nc) as tc:
        with tc.tile_pool(name="x8", bufs=KTILES) as px8, \
             tc.tile_pool(name="scr", bufs=3) as pscr, \
             tc.tile_pool(name="pers", bufs=1) as pers, \
             tc.tile_pool(name="simsb", bufs=4) as psim, \
             tc.tile_pool(name="simr", bufs=2) as psimr, \
             tc.tile_pool(name="mask", bufs=4) as pmask, \
             tc.tile_pool(name="sm", bufs=2) as psm, \
             tc.tile_pool(name="ps", bufs=2, space="PSUM") as pps:

            # ---- t0 warmups: CC mesh init + absrsqrt ACT table preload ----
            junk = pers.tile([128, 16], F32, tag="junk")
            nc.gpsimd.memset(junk[:], 1.0)
            nc.sync.dma_start(cc_warm_in[:], junk[0:1, :])
            nc.gpsimd.collective_compute(
                "AllReduce", ALU.add, replica_groups=grp,
                ins=[cc_warm_in[:].opt()], outs=[cc_warm_out[:].opt()])
            junk2 = pers.tile([128, 16], F32, tag="junk2")
            nc.scalar.activation(junk2[:], junk[:], AF.Abs_reciprocal_sqrt)

            # ---- load x (split DMAs over queues), partial ssq ----
            xb = []
            for k in range(KTILES):
                t = px8.tile([128, 2 * R], FP8, tag="x8")
                nc.sync.dma_start(t[0:64, :], x[k, 0:64, :])
                nc.sync.dma_start(t[64:128, :], x[k, 64:128, :])
                xb.append(t)

            # squares: 10 on ACT, 6 on DVE (fp8*fp8->bf16); adds in bf16 (2x DVE)
            accb = pers.tile([128, R], BF16, tag="accb")
            sq_prev = None
            n_sq = 0
            for k in range(KTILES):
                for s in range(2):
                    sq = pscr.tile([128, R], BF16, tag="scr")
                    src = xb[k][:, s * R:(s + 1) * R]
                    if n_sq % 8 < 5:
                        nc.scalar.activation(sq[:], src, AF.Square)
                    else:
                        nc.vector.tensor_tensor(sq[:], src, src, ALU.mult)
                    if n_sq == 0:
                        sq_prev = sq
                    elif n_sq == 1:
                        nc.vector.tensor_tensor(accb[:], sq_prev[:], sq[:], ALU.add)
                    else:
                        nc.vector.tensor_tensor(accb[:], accb[:], sq[:], ALU.add)
                    n_sq += 1

            for h in range(4):
                nc.sync.dma_start(cc_ssq_in[32 * h:32 * (h + 1), :],
                                  accb[32 * h:32 * (h + 1), :])
            nc.gpsimd.collective_compute(
                "AllReduce", ALU.add, replica_groups=grp,
                ins=[cc_ssq_in[:].opt()], outs=[cc_ssq_out[:].opt()])
            ssqr = pers.tile([128, R], BF16, tag="ssqr")
            for h in range(4):
                nc.sync.dma_start(ssqr[32 * h:32 * (h + 1), :],
                                  cc_ssq_out[32 * h:32 * (h + 1), :])

            # scale = S / sqrt(128 * ssq) = 1/sqrt(ssq * 128 / S^2), as fp8
            scale_f = pers.tile([128, R], F32, tag="scalef")
            nc.scalar.activation(scale_f[:], ssqr[:], AF.Abs_reciprocal_sqrt,
                                 scale=128.0 / (S * S))
            scale8 = pers.tile([128, R], FP8, tag="scale8")
            nc.vector.tensor_copy(scale8[:], scale_f[:])

            # ---- normalize in place: rn = x * scale (split DVE/GpSimd) ----
            n_nm = 0
            for k in range(KTILES):
                for s in range(2):
                    sl = xb[k][:, s * R:(s + 1) * R]
                    eng = nc.gpsimd if n_nm % 3 == 2 else nc.vector
                    eng.tensor_tensor(sl, sl, scale8[:], ALU.mult)
                    n_nm += 1

            # ---- gram partial, fp8 DoubleRow (K=256/inst) ----
            for mt in range(MT):
                ps = pps.tile([128, R], F32, tag="ps")
                for k in range(KTILES):
                    v = xb[k][:].rearrange("p (two n) -> p two n", two=2)
                    lhsT = v[:, :, mt * 128:(mt + 1) * 128]
                    for nch in range(4):
                        nc.tensor.matmul(
                            ps[:, nch * 512:(nch + 1) * 512],
                            lhsT,
                            v[:, :, nch * 512:(nch + 1) * 512],
                            start=(k == 0), stop=(k == KTILES - 1),
                            perf_mode=PM.DoubleRow)
                sb = psim.tile([128, R], BF16, tag="simsb")
                nc.vector.tensor_copy(sb[:], ps[:])
                ci, row = divmod(mt * 128, rs_rows)
                nc.sync.dma_start(cc_sim_in[ci][row:row + 64, :], sb[0:64, :])
                nc.sync.dma_start(cc_sim_in[ci][row + 64:row + 128, :],
                                  sb[64:128, :])

            # ---- chunked reduce-scatter of sim (bf16) ----
            for i in range(NRS):
                nc.gpsimd.collective_compute(
                    "ReduceScatter", ALU.add, replica_groups=grp,
                    ins=[cc_sim_in[i][:].opt()], outs=[cc_rs[i][:].opt()])

            # ---- loss on the owned rows (NRS x 64 = 256) ----
            mtiles = []
            for i in range(4):
                mt_ = pmask.tile([128, R], BF16, tag="mask")
                nc.sync.dma_start(mt_[:], masks[i, :, :])
                mtiles.append(mt_)

            ones = pers.tile([128, 1], F32, tag="ones")
            nc.vector.memset(ones[:], 1.0)
            loss_ps = pps.tile([1, 1], F32, tag="ps")

            for t in range(2):
                simr = psimr.tile([128, R], BF16, tag="simr")
                nc.sync.dma_start(simr[0:64, :], cc_rs[2 * t][:])
                nc.sync.dma_start(simr[64:128, :], cc_rs[2 * t + 1][:])

                ex = pscr.tile([128, R], F32, tag="scr")
                rowsum = psm.tile([128, 1], F32, tag="rowsum")
                nc.scalar.activation(ex[:], simr[:], AF.Exp, scale=INV_T_S2,
                                     accum_out=rowsum[:])

                scr1 = pscr.tile([128, R], BF16, tag="scrb")
                diag2 = psm.tile([128, 1], F32, tag="diag2")
                nc.vector.scalar_tensor_tensor(
                    scr1[:], simr[:], INV_T_S2, mtiles[t][:],
                    ALU.mult, ALU.mult, accum_out=diag2[:])

                scr2 = pscr.tile([128, R], BF16, tag="scrb")
                pos2 = psm.tile([128, 1], F32, tag="pos2")
                nc.vector.scalar_tensor_tensor(
                    scr2[:], simr[:], INV_T_S2, mtiles[2 + t][:],
                    ALU.mult, ALU.mult, accum_out=pos2[:])

                expdiag = psm.tile([128, 1], F32, tag="expdiag")
                nc.scalar.activation(expdiag[:], diag2[:], AF.Exp)
                den = psm.tile([128, 1], F32, tag="den")
                nc.vector.tensor_sub(den[:], rowsum[:], expdiag[:])
                lnden = psm.tile([128, 1], F32, tag="lnden")
                nc.scalar.activation(lnden[:], den[:], AF.Ln)
                losscol = psm.tile([128, 1], F32, tag="losscol")
                nc.vector.tensor_sub(losscol[:], lnden[:], pos2[:])

                nc.tensor.matmul(loss_ps[:], losscol[:], ones[:],
                                 start=(t == 0), stop=(t == 1))

            out_sb = pers.tile([1, 1], F32, tag="outsb")
            nc.vector.tensor_copy(out_sb[:], loss_ps[:])
            nc.sync.dma_start(y[:], out_sb[:])

    nc.compile()
    _CACHE["nc"] = nc
    return nc


def _rows_of_core(c):
    """Global row ids owned by core c, in loss-tile partition order."""
    p = np.arange(64)
    rows = []
    for ci in range(NRS):
        rows.append((R // NRS) * ci + 64 * c + p)
    return np.concatenate(rows)     # [256] = tiles [(0,1), (2,3)] halves


def _make_inputs(emb_i, emb_j):
    emb_i = np.asarray(emb_i, dtype=np.float32)
    emb_j = np.asarray(emb_j, dtype=np.float32)
    in_maps = []
    for c in range(NCORES):
        sl = slice(16 * c, 16 * (c + 1))
        xc = np.concatenate([emb_i[:, sl, :], emb_j[:, sl, :]], axis=0)
        # [r, m, n] -> [k, n, s, r] with m = 2k + s
        xc = xc.transpose(1, 2, 0).reshape(KTILES, 2, 128, R)
        xc = np.ascontiguousarray(xc.transpose(0, 2, 1, 3)).reshape(
            KTILES, 128, 2 * R).astype(ml_dtypes.float8_e4m3)
        masks = np.zeros((4, 128, R), dtype=np.float32)
        g = _rows_of_core(c)                        # [256]
        tt = np.arange(256) // 128                  # loss tile index
        pp = np.arange(256) % 128                   # partition in tile
        masks[tt, pp, g] = 1.0
        masks[2 + tt, pp, (g + B) % R] = 1.0
        in_maps.append({"x": xc, "masks": masks.astype(ml_dtypes.bfloat16)})
    return in_maps


def run(emb_i, emb_j, **spmd_kwargs):
    nc = _build_nc()
    in_maps = _make_inputs(emb_i, emb_j)
    res = bass_utils.run_bass_kernel_spmd(
        nc, in_maps, core_ids=list(range(NCORES)), **spmd_kwargs)
    total = sum(float(r["y"][0, 0]) for r in res.results)
    return np.array(total / R, dtype=np.float32), res


def kernel(emb_i, emb_j):
    loss, _ = run(emb_i, emb_j)
    return loss
